# revision 46
# baseline (speedup 1.0000x reference)
"""Multi-head cross-attention Trainium2 kernel (8-core SPMD, data-parallel).

Shards (batch=4) x (seq halves) across 8 NeuronCores; each core runs the
full q/kv/attention/out-proj pipeline for its 2048 query rows in bf16 with
fp32 PSUM accumulation.

Key tricks:
  - mask: reference adds +1.0 to logits of keys j < mask[b] before softmax.
    softmax(l + m) = e^m * e^l / sum  ->  fold e^m into V rows (and into the
    softmax-sum ones column), so masking costs nothing per tile.
  - softmax sums come from an extra ones column appended to V (head_dim 73);
    no vector reductions at all.
  - per-head K^T tiles are zero-padded to full 128-partition chunks so every
    matmul operand sits at base partition 0 (tile_position constraint).
  - normalization (1/sum) is applied after transposing per-head output to
    natural orientation, where rows are partitions and tensor_scalar works.

v2 changes vs baseline:
  - weights/activations stored partition-major in DRAM ([128, C, D]) so each
    tensor loads with one large DMA (contiguous per-partition runs) instead
    of 9; closes the ~21us startup bubble and cuts Sync-engine issue load.
  - dedicated PSUM tag budgeting: lps+qps (3 bufs), ops/onp/tps (3),
    yps/kps/vps (2) = 8 banks; removes logits stalls on psum recycling.
  - the 4 per-head 1/sum multiplies merged into one tensor_tensor with a
    stride-0 broadcast AP (onats is one [128, 4, D] tile).
  - oTc copies moved to DVE, ysb copies stay on ACT (out-proj phase was
    ACT-bound).
  - out-proj loops reordered c-outer so each oTc stationary is reused for
    all 3 f-chunks (1 LDWEIGHTS per c instead of 3).
  - output DMA split per 384-col chunk, pipelined behind the ysb copies.
"""

import sys

sys.path.insert(0, "/opt/trn_rl_repo")

import ml_dtypes
import numpy as np

import concourse.bass as bass  # noqa: F401  (engine types via nc)
import concourse.mybir as mybir
import concourse.tile as tile
from concourse import bacc
from concourse.bass_utils import run_bass_kernel_spmd
from concourse.masks import make_identity

BF16 = mybir.dt.bfloat16
F32 = mybir.dt.float32
NPBF16 = ml_dtypes.bfloat16
AF = mybir.ActivationFunctionType

B, NSEQ, MKEY, D, H, DH = 4, 4096, 300, 1152, 16, 72
NCORES = 8
C = D // 128  # 9 feature chunks
KC = 3  # key chunks, keys padded 300 -> 384
MP = KC * 128
RG = 512  # query rows per group
SCALE = 1.0 / float(np.sqrt(DH))
ROWS_PER_CORE = B * NSEQ // NCORES  # 2048

LAST_EXEC_NS = None
LAST_RESULT = None


def _head_segs(h):
    """Feature range [72h, 72h+72) of head h split at 128-chunk boundaries.

    Returns [(chunk, lo, hi)] with chunk-local partition range [lo, hi)."""
    f0, f1 = DH * h, DH * h + DH
    segs = []
    c = f0 // 128
    while c * 128 < f1:
        lo = max(f0, c * 128) - c * 128
        hi = min(f1, (c + 1) * 128) - c * 128
        segs.append((c, lo, hi))
        c += 1
    return segs


def _chunk_segs(c):
    """[(h, i, lo, hi)] head segments living in feature chunk c."""
    out = []
    for h in range(H):
        for i, (hc, lo, hi) in enumerate(_head_segs(h)):
            if hc == c:
                out.append((h, i, lo, hi))
    return out


# flat order of all (head, segment) pairs; column index into the hmask input
_ALL_SEGS = [(h, i) for h in range(H) for i in range(len(_head_segs(h)))]
_SEG_IDX = {hs: s for s, hs in enumerate(_ALL_SEGS)}
NSEG = len(_ALL_SEGS)


def _hmask_host():
    """[128, NSEG] f32: column (h,i) is 1.0 on the chunk-local partitions of
    that head segment, 0 elsewhere. Engine ops can't address SBUF at
    non-32-aligned partition bases, so head extraction is done as a
    full-chunk copy multiplied by this per-partition mask."""
    m = np.zeros((128, NSEG), np.float32)
    for h in range(H):
        for i, (_, lo, hi) in enumerate(_head_segs(h)):
            m[lo:hi, _SEG_IDX[(h, i)]] = 1.0
    return m


def build_program(rpc=ROWS_PER_CORE, has_bq=False, has_bk=False, has_bv=False, has_bp=False):
    nc = bacc.Bacc()

    groups = rpc // RG
    tiles_per_group = RG // 128
    kn = [128, 128, MKEY - 256]  # real keys per key chunk

    # partition-major layouts: one big DMA per tensor
    xT_d = nc.dram_tensor("xT", [128, groups, C, RG], BF16, kind="ExternalInput")
    condT_d = nc.dram_tensor("condT", [128, C, MKEY], BF16, kind="ExternalInput")
    # wq is output-chunk-major: wq[c][p][k][q] = Wq[k*128+p, c*128+q], so the
    # first q-proj column chain only waits for one 295KB DMA (plus xT)
    wq_d = nc.dram_tensor("wq", [C, 128, C, 128], BF16, kind="ExternalInput")
    wk_d = nc.dram_tensor("wk", [C, 128, C, 128], BF16, kind="ExternalInput")
    wv_d = nc.dram_tensor("wv", [128, C, D], BF16, kind="ExternalInput")
    wp_d = nc.dram_tensor("wp", [128, C, D], BF16, kind="ExternalInput")
    bq_d = nc.dram_tensor("bq", [128, C], F32, kind="ExternalInput")
    bk_d = nc.dram_tensor("bk", [128, C], F32, kind="ExternalInput")
    bv_d = nc.dram_tensor("bv", [1, D], BF16, kind="ExternalInput")
    bp_d = nc.dram_tensor("bp", [1, D], BF16, kind="ExternalInput")
    vs_d = nc.dram_tensor("vscale", [128, KC], F32, kind="ExternalInput")
    hm_d = nc.dram_tensor("hmask", [128, NSEG], F32, kind="ExternalInput")
    out_d = nc.dram_tensor("out", [rpc, D], BF16, kind="ExternalOutput")

    YCH = [(0, 384), (384, 768), (768, 1152)]

    with tile.TileContext(nc) as tc:
        with (
            tc.tile_pool(name="const", bufs=1) as cpool,
            tc.tile_pool(name="ps", bufs=3, space="PSUM") as psp,
        ):
            # persistent weights / constants (split big loads in thirds so the
            # first q-proj matmuls can start after ~1/3 of the bytes land)
            wq_sb = cpool.tile([128, C, D], BF16)
            wp_sb = cpool.tile([128, C, D], BF16)
            ident = cpool.tile([128, 128], BF16)
            make_identity(nc, ident[:])
            # vs/hm are tiny and first needed in the kv phase; issue their
            # DMAs after the startup-critical wq/xT loads (see q_proj)
            vs_sb = cpool.tile([128, KC], F32)
            hm_sb = cpool.tile([128, NSEG], F32)
            if has_bq:
                bq_sb = cpool.tile([128, C], F32)
                nc.sync.dma_start(bq_sb[:], bq_d[:])
            if has_bk:
                bk_sb = cpool.tile([128, C], F32)
                nc.sync.dma_start(bk_sb[:], bk_d[:])
            if has_bp:
                bp_sb = cpool.tile([1, D], BF16)
                nc.sync.dma_start(bp_sb[:], bp_d[:])
            if has_bv or has_bp:
                ones_sb = cpool.tile([1, 128], BF16)
                nc.gpsimd.memset(ones_sb[:], 1.0)

            # V in natural orientation [key, head, dim+1]; fake keys stay 0,
            # col 72 holds e^mask (ones column pre-scaled by the mask factor)
            v_sb = cpool.tile([128, KC, H, DH + 1], BF16)
            nc.gpsimd.memset(v_sb[:], 0.0)
            kTz = {}
            for h in range(H):
                for i in range(len(_head_segs(h))):
                    t = cpool.tile([128, MP], BF16, name=f"kTz_{h}_{i}")
                    nc.gpsimd.memset(t[:], 0.0)
                    kTz[(h, i)] = t

            kT72 = {}
            for h in range(H):
                if len(_head_segs(h)) == 2:
                    t = cpool.tile([DH + 4, MP], BF16, name=f"kT72_{h}")
                    nc.gpsimd.memset(t[:], 0.0)
                    kT72[h] = t

            # ---- streaming: q-proj, attention, out-proj ----
            with tc.tile_pool(name="xq", bufs=2) as xqpool:
                qts = {}
                first = [True]

                def q_proj(g):
                    xT_sb = xqpool.tile([128, C, RG], BF16, name="xT", tag="xT")
                    if first[0]:
                        # startup: first q-proj output chunk is gated only by
                        # wq col-chunk 0 and xT; remaining wq chunks stream in
                        nc.sync.dma_start(wq_sb[:, :, 0:128], wq_d[0])
                        for k0, k1 in [(0, 3), (3, 6), (6, C)]:
                            nc.sync.dma_start(xT_sb[:, k0:k1, :], xT_d[:, g, k0:k1, :])
                        for c in range(1, C):
                            nc.sync.dma_start(
                                wq_sb[:, :, c * 128 : (c + 1) * 128], wq_d[c]
                            )
                        nc.sync.dma_start(vs_sb[:], vs_d[:])
                        nc.sync.dma_start(hm_sb[:], hm_d[:])
                        first[0] = False
                    else:
                        nc.sync.dma_start(xT_sb[:], xT_d[:, g])
                    qT_sb = xqpool.tile([128, C, RG], BF16, name="qT", tag="qT")
                    for c in range(C):
                        qps = psp.tile([128, RG], F32, name="qps", tag="big", bufs=4)
                        for k in range(C):
                            nc.tensor.matmul(
                                qps[:],
                                wq_sb[:, k, c * 128 : (c + 1) * 128],
                                xT_sb[:, k, :],
                                start=(k == 0),
                                stop=(k == C - 1),
                            )
                        if has_bq:
                            nc.scalar.activation(
                                qT_sb[:, c, :], qps[:], AF.Identity, bias=bq_sb[:, c : c + 1]
                            )
                        else:
                            nc.vector.tensor_copy(qT_sb[:, c, :], qps[:])
                    qts[g] = qT_sb

                q_proj(0)
                # ---- kv projection (weights in a scoped SBUF pool) ----
                with tc.tile_pool(name="kvw", bufs=1) as kvpool:
                    condT_sb = kvpool.tile([128, C, MKEY], BF16)
                    wk_sb = kvpool.tile([128, C, D], BF16)
                    wv_sb = kvpool.tile([128, C, D], BF16)
                    nc.sync.dma_start(condT_sb[:], condT_d[:])
                    # interleave wk (by output chunk) and wv (by vch column
                    # group) so the kv K/V chains start as bytes land
                    vch_bounds = [(0, 360), (360, 720), (720, 1080), (1080, 1152)]
                    for c in range(C):
                        nc.sync.dma_start(wk_sb[:, :, c * 128 : (c + 1) * 128], wk_d[c])
                        if c < len(vch_bounds):
                            f0, f1 = vch_bounds[c]
                            nc.sync.dma_start(wv_sb[:, :, f0:f1], wv_d[:, :, f0:f1])
                    nc.sync.dma_start(wp_sb[:], wp_d[:])
                    if has_bv:
                        bv_sb = kvpool.tile([1, D], BF16)
                        nc.sync.dma_start(bv_sb[:], bv_d[:])

                    # K^T in feature-chunk orientation -> zero-padded head tiles
                    def emit_k(c):
                        kps = psp.tile([128, MKEY], F32, name="kps", tag="yk", bufs=2)
                        for k in range(C):
                            nc.tensor.matmul(
                                kps[:],
                                wk_sb[:, k, c * 128 : (c + 1) * 128],
                                condT_sb[:, k, :],
                                start=(k == 0),
                                stop=(k == C - 1),
                            )
                        for h, i, _lo, _hi in _chunk_segs(c):
                            s = _SEG_IDX[(h, i)]
                            if has_bk:
                                nc.vector.tensor_scalar(
                                    kTz[(h, i)][:, 0:MKEY],
                                    kps[:],
                                    bk_sb[:, c : c + 1],
                                    hm_sb[:, s : s + 1],
                                    op0=mybir.AluOpType.add,
                                    op1=mybir.AluOpType.mult,
                                )
                            else:
                                nc.vector.tensor_scalar_mul(
                                    kTz[(h, i)][:, 0:MKEY], kps[:], hm_sb[:, s : s + 1]
                                )

                    # V natural [keys, feat], head-aligned 360-wide chunks
                    vch = [(0, 360), (360, 720), (720, 1080), (1080, 1152)]

                    def emit_v(kc, f0, f1):
                        vps = psp.tile([128, f1 - f0], F32, name="vps", tag="yk", bufs=2)
                        for k in range(C):
                            nc.tensor.matmul(
                                vps[0 : kn[kc], :],
                                condT_sb[:, k, kc * 128 : kc * 128 + kn[kc]],
                                wv_sb[:, k, f0:f1],
                                start=(k == 0),
                                stop=(k == C - 1 and not has_bv),
                            )
                        if has_bv:
                            nc.tensor.matmul(
                                vps[0 : kn[kc], :],
                                ones_sb[0:1, 0 : kn[kc]],
                                bv_sb[0:1, f0:f1],
                                start=False,
                                stop=True,
                            )
                        for h in range(f0 // DH, f1 // DH):
                            d0 = h * DH - f0
                            nc.vector.tensor_scalar_mul(
                                v_sb[0 : kn[kc], kc, h, 0:DH],
                                vps[0 : kn[kc], d0 : d0 + DH],
                                vs_sb[0 : kn[kc], kc : kc + 1],
                            )

                    # interleave K and V chunks: the PE streams one chain
                    # while the DVE drains the other's extraction muls
                    vlist = [(kc, f0, f1) for kc in range(KC) for (f0, f1) in vch]
                    for j in range(max(C, len(vlist))):
                        if j < C:
                            emit_k(j)
                        if j < len(vlist):
                            emit_v(*vlist[j])
                    for kc in range(KC):
                        for h in range(H):
                            nc.any.tensor_copy(
                                v_sb[0 : kn[kc], kc, h, DH : DH + 1],
                                vs_sb[0 : kn[kc], kc : kc + 1],
                            )

                    for h, t in kT72.items():
                        (c0, lo0, hi0), (c1, lo1, hi1) = _head_segs(h)
                        n0 = hi0 - lo0
                        nc.sync.dma_start(t[0:n0, 0:MKEY], kTz[(h, 0)][lo0:hi0, 0:MKEY])
                        nc.sync.dma_start(t[n0 : n0 + (hi1 - lo1), 0:MKEY], kTz[(h, 1)][lo1:hi1, 0:MKEY])
                with (
                    tc.tile_pool(name="att", bufs=4) as apool,
                    tc.tile_pool(name="outp", bufs=2) as opool,
                ):
                    def attention(g):
                        # the last group's out-proj has no successor work to
                        # hide psum-evacuation waits; borrow the (idle) lps/qps
                        # slots for deeper yps pipelining there
                        ytag, ybufs = ("big", 4) if g == groups - 1 else ("yk", 2)
                        qT_sb = qts.pop(g)
                        qTg = {}
                        for h in sorted(kT72):
                            (c0, lo0, hi0), (c1, lo1, hi1) = _head_segs(h)
                            n0 = hi0 - lo0
                            t = opool.tile([DH + 4, RG], BF16, name=f"qTg{h}", tag=f"qTg{h}")
                            nc.sync.dma_start(t[0:n0, :], qT_sb[lo0:hi0, c0, :])
                            nc.sync.dma_start(t[n0:DH, :], qT_sb[lo1:hi1, c1, :])
                            qTg[h] = t

                        # one [128, rt, feat] natural-orientation tile per group
                        onat = opool.tile(
                            [128, tiles_per_group, D], BF16, name="onat", tag="onat"
                        )
                        head_order = [h for h in range(H) if h not in kT72] + sorted(kT72)

                        def head_pv(h, expT):
                            """PV + psum evacuation for head h (one head behind
                            the logits/exp front)."""
                            ops = psp.tile([DH + 1, RG], F32, name="ops", tag="att", bufs=2)
                            for kc in range(KC):
                                nc.tensor.matmul(
                                    ops[:],
                                    v_sb[:, kc, h, :],
                                    expT[:, kc, :],
                                    start=(kc == 0),
                                    stop=(kc == KC - 1),
                                )
                            oT_sb = apool.tile([DH + 1, RG], BF16, name="oT", tag="oT")
                            nc.vector.tensor_copy(oT_sb[:], ops[:])
                            return oT_sb

                        def head_norm(h, oT_sb):
                            """transpose + normalize for head h (two heads
                            behind, so the oT cast has a full head-period)."""
                            onp = psp.tile(
                                [128, tiles_per_group, DH + 4], BF16, name="onp", tag="att", bufs=2
                            )
                            for rt in range(tiles_per_group):
                                nc.tensor.transpose(
                                    onp[:, rt, 0 : DH + 1],
                                    oT_sb[:, rt * 128 : (rt + 1) * 128],
                                    ident[0 : DH + 1, 0 : DH + 1],
                                )
                            inv = apool.tile([128, tiles_per_group], F32, name="inv", tag="inv")
                            nc.vector.reciprocal(inv[:], onp[:, :, DH])
                            # all 4 row-tiles normalized in one op (stride-0
                            # broadcast of inv along the feature dim)
                            nc.vector.tensor_mul(
                                onat[:, :, h * DH : (h + 1) * DH],
                                onp[:, :, 0:DH],
                                inv[:].unsqueeze(2).broadcast_to([128, tiles_per_group, DH]),
                            )

                        p1 = p2 = None
                        for h in head_order:
                            segs = _head_segs(h)
                            expT = apool.tile([128, KC, RG], BF16, name="expT", tag="expT")
                            for kc in range(KC):
                                lps = psp.tile([128, RG], F32, name="lps", tag="big", bufs=4)
                                if h in kT72:
                                    nc.tensor.matmul(
                                        lps[:],
                                        kT72[h][0:DH, kc * 128 : (kc + 1) * 128],
                                        qTg[h][0:DH, :],
                                        start=True,
                                        stop=True,
                                    )
                                else:
                                    (c, lo, hi) = segs[0]
                                    nc.tensor.matmul(
                                        lps[:],
                                        kTz[(h, 0)][:, kc * 128 : (kc + 1) * 128],
                                        qT_sb[:, c, :],
                                        start=True,
                                        stop=True,
                                    )
                                nc.scalar.activation(expT[:, kc, :], lps[:], AF.Exp, scale=SCALE)
                            if p1 is not None:
                                oT = head_pv(*p1)
                                if p2 is not None:
                                    head_norm(*p2)
                                p2 = (p1[0], oT)
                            p1 = (h, expT)
                        oT = head_pv(*p1)
                        head_norm(*p2)
                        head_norm(p1[0], oT)

                        for rt in range(tiles_per_group):
                            grt = g * tiles_per_group + rt
                            oTc_sb = opool.tile([128, C, 128], BF16, name="oTc", tag="oTc")
                            for c3 in range(C // 3):
                                tps = psp.tile([128, 3, 128], BF16, name="tps", tag="att", bufs=2)
                                for j in range(3):
                                    c = c3 * 3 + j
                                    nc.tensor.transpose(
                                        tps[:, j, :],
                                        onat[:, rt, c * 128 : (c + 1) * 128],
                                        ident[:],
                                    )
                                nc.vector.tensor_copy(oTc_sb[:, c3 * 3 : c3 * 3 + 3, :], tps[:])

                            ysb = opool.tile([128, D], BF16, name="ysb", tag="y")
                            ypss = [
                                psp.tile([128, f1 - f0], F32, name=f"yps{fi}", tag=ytag, bufs=ybufs)
                                for fi, (f0, f1) in enumerate(YCH[:2])
                            ]
                            # c-outer: each oTc stationary reused for 2 f-chunks
                            for c in range(C):
                                for fi, (f0, f1) in enumerate(YCH[:2]):
                                    nc.tensor.matmul(
                                        ypss[fi][:],
                                        oTc_sb[:, c, :],
                                        wp_sb[:, c, f0:f1],
                                        start=(c == 0),
                                        stop=(c == C - 1 and not has_bp),
                                    )
                            for fi, (f0, f1) in enumerate(YCH[:2]):
                                if has_bp:
                                    nc.tensor.matmul(
                                        ypss[fi][:],
                                        ones_sb[0:1, :],
                                        bp_sb[0:1, f0:f1],
                                        start=False,
                                        stop=True,
                                    )
                                # alternate psum-evacuation engines so yps
                                # recycling isn't gated on one engine
                                if fi == 1:
                                    nc.scalar.copy(ysb[:, f0:f1], ypss[fi][:])
                                else:
                                    nc.vector.tensor_copy(ysb[:, f0:f1], ypss[fi][:])
                                nc.sync.dma_start(
                                    out_d[grt * 128 : (grt + 1) * 128, f0:f1], ysb[:, f0:f1]
                                )
                            f0, f1 = YCH[2]
                            yps2 = psp.tile([128, f1 - f0], F32, name="yps2", tag=ytag, bufs=ybufs)
                            for c in range(C):
                                nc.tensor.matmul(
                                    yps2[:],
                                    oTc_sb[:, c, :],
                                    wp_sb[:, c, f0:f1],
                                    start=(c == 0),
                                    stop=(c == C - 1 and not has_bp),
                                )
                            if has_bp:
                                nc.tensor.matmul(
                                    yps2[:],
                                    ones_sb[0:1, :],
                                    bp_sb[0:1, f0:f1],
                                    start=False,
                                    stop=True,
                                )
                            nc.vector.tensor_copy(ysb[:, f0:f1], yps2[:])
                            nc.sync.dma_start(
                                out_d[grt * 128 : (grt + 1) * 128, f0:f1], ysb[:, f0:f1]
                            )
                    for g in range(groups):
                        if g + 1 < groups:
                            q_proj(g + 1)
                        attention(g)

    nc.compile()
    return nc


_programs = {}


def _get_program(key):
    if key not in _programs:
        _programs[key] = build_program(*key)
    return _programs[key]


def make_in_maps(x, cond, mask, Wq, bq, Wkv, bkv, Wp, bp, rpc=ROWS_PER_CORE, ncores=NCORES):
    """Host-side shard + relayout. Returns (in_maps, flags)."""
    x = np.asarray(x, np.float32)
    cond = np.asarray(cond, np.float32)
    mask = np.asarray(mask)
    Wq = np.asarray(Wq, np.float32)
    Wkv = np.asarray(Wkv, np.float32)
    Wp = np.asarray(Wp, np.float32)
    bq = np.asarray(bq, np.float32)
    bkv = np.asarray(bkv, np.float32)
    bp = np.asarray(bp, np.float32)

    def pmajor(w):  # [D, D2] -> [128, C, D2] partition-major
        d2 = w.shape[1]
        return np.ascontiguousarray(
            w.astype(NPBF16).reshape(C, 128, d2).transpose(1, 0, 2)
        )

    # [C_out, 128, C_in, 128]: wq[c, p, k, q] = Wq[k*128+p, c*128+q]
    wq = np.ascontiguousarray(
        Wq.astype(NPBF16).reshape(C, 128, C, 128).transpose(2, 1, 0, 3)
    )
    wk = np.ascontiguousarray(
        Wkv[:, :D].astype(NPBF16).reshape(C, 128, C, 128).transpose(2, 1, 0, 3)
    )
    wv = pmajor(Wkv[:, D:])
    wp = pmajor(Wp)
    bq_a = np.ascontiguousarray(bq.reshape(C, 128).T)
    bk_a = np.ascontiguousarray(bkv[:D].reshape(C, 128).T)
    bv_a = bkv[D:].astype(NPBF16).reshape(1, D)
    bp_a = bp.astype(NPBF16).reshape(1, D)

    flags = (rpc, bool(bq.any()), bool(bkv[:D].any()), bool(bkv[D:].any()), bool(bp.any()))
    hmask = _hmask_host()

    halves = NSEQ // rpc
    groups = rpc // RG
    in_maps = []
    for core in range(ncores):
        b, half = core // halves, core % halves
        rows = slice(half * rpc, (half + 1) * rpc)
        # [128, G, C, RG]: xT[p, g, c, r] = x[b, g*RG+r, c*128+p]
        xT = np.ascontiguousarray(
            x[b, rows].T.astype(NPBF16).reshape(C, 128, groups, RG).transpose(1, 2, 0, 3)
        )
        condT = np.ascontiguousarray(
            cond[b].T.astype(NPBF16).reshape(C, 128, MKEY).transpose(1, 0, 2)
        )
        mv = (np.arange(MP) < int(mask[b])).astype(np.float32)
        vscale = np.ascontiguousarray(np.exp(mv).reshape(KC, 128).T)
        in_maps.append(
            {
                "xT": xT,
                "condT": condT,
                "wq": wq,
                "wk": wk,
                "wv": wv,
                "wp": wp,
                "bq": bq_a,
                "bk": bk_a,
                "bv": bv_a,
                "bp": bp_a,
                "vscale": vscale,
                "hmask": hmask,
            }
        )
    return in_maps, flags


def kernel(x, cond, mask, Wq, bq, Wkv, bkv, Wp, bp):
    global LAST_EXEC_NS
    import os
    import time

    in_maps, flags = make_in_maps(x, cond, mask, Wq, bq, Wkv, bkv, Wp, bp)
    nc = _get_program(flags)
    trace = bool(os.environ.get("BASS_KERNEL_TRACE"))
    res = None
    for attempt in range(3):
        try:
            res = run_bass_kernel_spmd(nc, in_maps, list(range(NCORES)), trace=trace)
            break
        except Exception:
            if attempt == 2:
                raise
            time.sleep(10)
    LAST_EXEC_NS = res.exec_time_ns
    globals()["LAST_RESULT"] = res

    rpc = flags[0]
    halves = NSEQ // rpc
    out = np.empty((B, NSEQ, D), np.float32)
    for core in range(NCORES):
        b, half = core // halves, core % halves
        out[b, half * rpc : (half + 1) * rpc] = res.results[core]["out"].astype(np.float32)
    return out
